# revision 10
# baseline (speedup 1.0000x reference)
"""Masked multi-head self-attention for Trainium2, SPMD over 8 NeuronCores.

Sharding: core c handles batch c//2, query-half c%2 (1024 of 2048 query rows).
The same Bass program runs on every core; odd cores get their inputs rotated
along the key axis so that "my" query rows are always tokens [0, 1024)
(attention sums are invariant to a consistent permutation of the key axis).

Host supplies x^T (features-major q) and (1-mask)^T in fp16, so the device
does no transposes. Per-core dataflow:
  Q^T/K^T (head-pair packed, fp32r) and V (token-major fp16, with a ones
        column for the softmax denominator) via PE projections from x^T
  S^T   = K @ Q^T per (head, key-tile) into fp32 PSUM (fp32r operands)
  U     = exp(0.125 * S^T - 8) on ACT (PSUM -> SBUF fp16), masked by
        (1-mask)^T via one DVE multiply
  headsT + denominator via PE (U moving, [V | 1] stationary, fp16)
  normalize straight out of PSUM per head-pair (reciprocal + GPSIMD
        partition-broadcast + DVE multiply -> fp32r headsT)
  output projection (fp32r) accumulated over heads in PSUM.
"""

import sys

sys.path.insert(0, "/opt/trn_rl_repo")

import numpy as np

import concourse.bass as bass  # noqa: F401
import concourse.tile as tile
from concourse import bacc, mybir
from concourse.bass_utils import run_bass_kernel_spmd

F32 = mybir.dt.float32
F32R = mybir.dt.float32r
F16 = mybir.dt.float16
EXP = mybir.ActivationFunctionType.Exp

B, N, D, H, DK = 4, 2048, 512, 8, 64
NQ = N // 2          # query rows per core
NORM = 1.0 / 8.0     # 1/sqrt(DK)
NFC = D // 128       # feature chunks (4)
NHP = H // 2         # head pairs (4)
NGT = N // 128       # key tiles (16)
NQT = NQ // 128      # query tiles per core (8)
NCORES = 8

_CACHE = {}


def _build():
    if "nc" in _CACHE:
        return _CACHE["nc"]
    nc = bacc.Bacc("TRN2", target_bir_lowering=False, debug=False,
                   num_devices=NCORES)
    xqt = nc.dram_tensor("xqt", [D, N], F32, kind="ExternalInput")
    nmtd = nc.dram_tensor("nmt", [N, NQ], F16, kind="ExternalInput")
    wq = nc.dram_tensor("wq", [D, D], F32, kind="ExternalInput")
    wk = nc.dram_tensor("wk", [D, D], F32, kind="ExternalInput")
    wv = nc.dram_tensor("wv", [D, D], F32, kind="ExternalInput")
    wo = nc.dram_tensor("wo", [DK, H * D], F32, kind="ExternalInput")
    out = nc.dram_tensor("out", [NQ, D], F32, kind="ExternalOutput")

    with tile.TileContext(nc) as tc:
        with tc.tile_pool(name="persist", bufs=1) as P:
            kt = P.tile([128, NHP, N], F32R)    # K^T two heads per tile
            qt_ = P.tile([128, NHP, NQ], F32R)  # Q^T two heads per tile
            v_ = P.tile([128, NGT, H, DK + 1], F16)  # V | ones
            nmt = P.tile([128, NGT, NQ], F16)        # (1-mask)^T
            nbias = P.tile([128, 1], F32)
            nc.vector.memset(nbias[:], -8.0)
            nc.vector.memset(v_[:, :, :, DK:DK + 1], 1.0)
            nmtv = nmtd.rearrange("(gc p) q -> p gc q", p=128)
            for gc in range(NGT):
                nc.sync.dma_start(out=nmt[:, gc, :], in_=nmtv[:, gc, :])

            # ---------------- phase A: loads + projections ----------------
            with tc.tile_pool(name="paps", bufs=3, space="PSUM") as APs, \
                 tc.tile_pool(name="xtp", bufs=1) as XT:
                xt = XT.tile([128, NFC, N], F32R)
                wqb = XT.tile([128, NFC, D], F32R)
                wkb = XT.tile([128, NFC, D], F32R)
                wvb = XT.tile([128, NFC, D], F32R)
                for fc in range(NFC):
                    for dram, dst in ((wq, wqb), (wk, wkb), (wv, wvb)):
                        nc.sync.dma_start(
                            out=dst[:, fc, :],
                            in_=dram[fc * 128:(fc + 1) * 128, :].bitcast(F32R))
                # x^T loaded per (token-group, chunk) so projections start
                # before the full tensor lands
                for ttg in range(4):
                    for fc in range(NFC):
                        nc.sync.dma_start(
                            out=xt[:, fc, ttg * 512:(ttg + 1) * 512],
                            in_=xqt[fc * 128:(fc + 1) * 128,
                                    ttg * 512:(ttg + 1) * 512].bitcast(F32R))

                def proj_q(hp):
                    for ttg in range(2):
                        ps = APs.tile([128, 512], F32, tag="projps",
                                      name="psq")
                        for fc in range(NFC):
                            nc.tensor.matmul(
                                ps[:],
                                wqb[:, fc, hp * 128:(hp + 1) * 128],
                                xt[:, fc, ttg * 512:(ttg + 1) * 512],
                                start=(fc == 0), stop=(fc == NFC - 1))
                        nc.vector.tensor_copy(
                            qt_[:, hp, ttg * 512:(ttg + 1) * 512], ps[:])

                def proj_k(hp):
                    for ttg in range(4):
                        ps = APs.tile([128, 512], F32, tag="projps",
                                      name="psk")
                        for fc in range(NFC):
                            nc.tensor.matmul(
                                ps[:],
                                wkb[:, fc, hp * 128:(hp + 1) * 128],
                                xt[:, fc, ttg * 512:(ttg + 1) * 512],
                                start=(fc == 0), stop=(fc == NFC - 1))
                        nc.vector.tensor_copy(
                            kt[:, hp, ttg * 512:(ttg + 1) * 512], ps[:])

                proj_q(0)
                proj_k(0)
                for gt in range(NGT):
                    ps = APs.tile([128, 512], F32, tag="projps", name="psv")
                    for fc in range(NFC):
                        nc.tensor.matmul(
                            ps[:],
                            xt[:, fc, gt * 128:(gt + 1) * 128],
                            wvb[:, fc, :],
                            start=(fc == 0), stop=(fc == NFC - 1))
                    nc.vector.tensor_copy(
                        v_[:, gt, :, 0:DK],
                        ps.rearrange("p (h v) -> p h v", h=H))
                for hp in range(1, NHP):
                    proj_q(hp)
                    proj_k(hp)

            with tc.tile_pool(name="late", bufs=1) as L:
                htn = L.tile([DK, H, NQ], F32R)   # normalized headsT
                dsum = L.tile([1, H, NQ], F32)    # 1/denominator rows
                wob = L.tile([DK, H * D], F32R)
                nc.sync.dma_start(out=wob[:], in_=wo[:, :].bitcast(F32R))

                # ---------------- phase B ----------------
                with tc.tile_pool(name="ub", bufs=4) as UB, \
                     tc.tile_pool(name="nrm", bufs=2) as NR, \
                     tc.tile_pool(name="spsp", bufs=2, space="PSUM") as SPs, \
                     tc.tile_pool(name="hvp", bufs=1, space="PSUM") as HVs:
                    for hp in range(NHP):
                        hv = [HVs.tile([DK + 1, NQ], F32, tag=f"hv{i}",
                                       name=f"hv{i}") for i in range(2)]
                        for gt in range(NGT):
                            for i in range(2):
                                h = hp * 2 + i
                                s = SPs.tile([128, NQ], F32, tag="sps")
                                for qg in range(2):
                                    nc.tensor.matmul(
                                        s[:, qg * 512:(qg + 1) * 512],
                                        kt[i * 64:(i + 1) * 64, hp,
                                           gt * 128:(gt + 1) * 128],
                                        qt_[i * 64:(i + 1) * 64, hp,
                                            qg * 512:(qg + 1) * 512],
                                        start=True, stop=True)
                                u = UB.tile([128, NQ], F16, tag="u")
                                nc.scalar.activation(u[:], s[:], EXP,
                                                     bias=nbias[:],
                                                     scale=NORM)
                                nc.vector.tensor_mul(u[:], u[:], nmt[:, gt, :])
                                for qg in range(2):
                                    nc.tensor.matmul(
                                        hv[i][:, qg * 512:(qg + 1) * 512],
                                        v_[:, gt, h, :],
                                        u[:, qg * 512:(qg + 1) * 512],
                                        start=(gt == 0), stop=(gt == NGT - 1))
                        # normalize straight out of PSUM
                        for i in range(2):
                            h = hp * 2 + i
                            nc.scalar.copy(dsum[0:1, h, :],
                                           hv[i][DK:DK + 1, :])
                            nc.vector.reciprocal_approx_fast(
                                dsum[0:1, h, :], dsum[0:1, h, :])
                            rinvb = NR.tile([DK, NQ], F32, tag="rinvb")
                            nc.gpsimd.partition_broadcast(rinvb[:],
                                                          dsum[0:1, h, :])
                            nc.vector.tensor_mul(htn[:, h, :],
                                                 hv[i][0:DK, :], rinvb[:])

                # ---------------- phase D: output projection ----------------
                with tc.tile_pool(name="pd", bufs=2) as DP, \
                     tc.tile_pool(name="pdps", bufs=2, space="PSUM") as DPs:
                    for qt in range(NQT):
                        po = DPs.tile([128, 512], F32, tag="po")
                        for h in range(H):
                            nc.tensor.matmul(
                                po[:],
                                htn[:, h, qt * 128:(qt + 1) * 128],
                                wob[:, h * D:(h + 1) * D],
                                start=(h == 0), stop=(h == H - 1))
                        ob = DP.tile([128, 512], F32, tag="ob")
                        nc.vector.tensor_copy(ob[:], po[:])
                        nc.sync.dma_start(
                            out=out[qt * 128:(qt + 1) * 128, :], in_=ob[:])

    nc.compile()
    _CACHE["nc"] = nc
    return nc


def kernel(q, mask, W_query, W_key, W_val, W_out):
    q = np.asarray(q, dtype=np.float32)
    mask = np.asarray(mask, dtype=np.int32)
    # [f, h*64+k] layouts for the projections, [k, h*512+e] for the output
    wq_r = np.ascontiguousarray(
        np.transpose(np.asarray(W_query, np.float32), (1, 0, 2)).reshape(D, D))
    wk_r = np.ascontiguousarray(
        np.transpose(np.asarray(W_key, np.float32), (1, 0, 2)).reshape(D, D))
    wv_r = np.ascontiguousarray(
        np.transpose(np.asarray(W_val, np.float32), (1, 0, 2)).reshape(D, D))
    wo_r = np.ascontiguousarray(
        np.transpose(np.asarray(W_out, np.float32), (1, 0, 2)).reshape(DK, H * D))

    nc = _build()
    in_maps = []
    for c in range(NCORES):
        b, qh = c // 2, c % 2
        xqt_c = q[b].T                                      # (D, N)
        nmt_c = 1.0 - mask[b, qh * NQ:(qh + 1) * NQ, :].T   # (N, NQ)
        if qh:
            # rotate the key axis so this core's queries are tokens [0, NQ)
            xqt_c = np.roll(xqt_c, -NQ, axis=1)
            nmt_c = np.roll(nmt_c, -NQ, axis=0)
        in_maps.append({
            "xqt": np.ascontiguousarray(xqt_c),
            "nmt": np.ascontiguousarray(nmt_c.astype(np.float16)),
            "wq": wq_r, "wk": wk_r, "wv": wv_r, "wo": wo_r,
        })
    res = run_bass_kernel_spmd(nc, in_maps, core_ids=list(range(NCORES)))
    output = np.empty((B, N, D), np.float32)
    for c in range(NCORES):
        b, qh = c // 2, c % 2
        output[b, qh * NQ:(qh + 1) * NQ, :] = res.results[c]["out"]
    return output


# revision 11
# speedup vs baseline: 1.0214x; 1.0214x over previous
"""Masked multi-head self-attention for Trainium2, SPMD over 8 NeuronCores.

Sharding: core c handles batch c//2, query-half c%2 (1024 of 2048 query rows).
The same Bass program runs on every core; odd cores get their inputs rotated
along the key axis so that "my" query rows are always tokens [0, 1024)
(attention sums are invariant to a consistent permutation of the key axis).

Host supplies x^T (features-major q) and (1-mask)^T in fp16, so the device
does no transposes. Per-core dataflow:
  Q^T/K^T (head-pair packed, fp32r) and V (token-major fp16, with a ones
        column for the softmax denominator) via PE projections from x^T
  S^T   = K @ Q^T per (head, key-tile) into fp32 PSUM (fp32r operands)
  U     = exp(0.125 * S^T - 8) on ACT (PSUM -> SBUF fp16), masked by
        (1-mask)^T via one DVE multiply
  headsT + denominator via PE (U moving, [V | 1] stationary, fp16)
  normalize straight out of PSUM per head-pair (reciprocal + GPSIMD
        partition-broadcast + DVE multiply -> fp32r headsT)
  output projection (fp32r) accumulated over heads in PSUM.
"""

import sys

sys.path.insert(0, "/opt/trn_rl_repo")

import numpy as np

import concourse.bass as bass  # noqa: F401
import concourse.tile as tile
from concourse import bacc, mybir
from concourse.bass_utils import run_bass_kernel_spmd

F32 = mybir.dt.float32
F32R = mybir.dt.float32r
F16 = mybir.dt.float16
EXP = mybir.ActivationFunctionType.Exp

B, N, D, H, DK = 4, 2048, 512, 8, 64
NQ = N // 2          # query rows per core
NORM = 1.0 / 8.0     # 1/sqrt(DK)
NFC = D // 128       # feature chunks (4)
NHP = H // 2         # head pairs (4)
NGT = N // 128       # key tiles (16)
NQT = NQ // 128      # query tiles per core (8)
NCORES = 8

_CACHE = {}


def _build():
    if "nc" in _CACHE:
        return _CACHE["nc"]
    nc = bacc.Bacc("TRN2", target_bir_lowering=False, debug=False,
                   num_devices=NCORES)
    xqt = nc.dram_tensor("xqt", [D, N], F32, kind="ExternalInput")
    nmtd = nc.dram_tensor("nmt", [N, NQ], F16, kind="ExternalInput")
    wq = nc.dram_tensor("wq", [D, D], F32, kind="ExternalInput")
    wk = nc.dram_tensor("wk", [D, D], F32, kind="ExternalInput")
    wv = nc.dram_tensor("wv", [D, D], F32, kind="ExternalInput")
    wo = nc.dram_tensor("wo", [DK, H * D], F32, kind="ExternalInput")
    out = nc.dram_tensor("out", [NQ, D], F32, kind="ExternalOutput")

    with tile.TileContext(nc) as tc:
        with tc.tile_pool(name="persist", bufs=1) as P:
            kt = P.tile([128, NHP, N], F32R)    # K^T two heads per tile
            qt_ = P.tile([128, NHP, NQ], F32R)  # Q^T two heads per tile
            v_ = P.tile([128, NGT, H, DK + 1], F16)  # V | ones
            nmt = P.tile([128, NGT, NQ], F16)        # (1-mask)^T
            nbias = P.tile([128, 1], F32)
            nc.vector.memset(nbias[:], -8.0)
            nc.vector.memset(v_[:, :, :, DK:DK + 1], 1.0)
            nmtv = nmtd.rearrange("(gc p) q -> p gc q", p=128)
            for gc in range(NGT):
                nc.sync.dma_start(out=nmt[:, gc, :], in_=nmtv[:, gc, :])

            # ---------------- phase A: loads + projections ----------------
            with tc.tile_pool(name="paps", bufs=3, space="PSUM") as APs, \
                 tc.tile_pool(name="xtp", bufs=1) as XT:
                xt = XT.tile([128, NFC, N], F32R)
                wqb = XT.tile([128, NFC, D], F32R)
                wkb = XT.tile([128, NFC, D], F32R)
                wvb = XT.tile([128, NFC, D], F32R)
                def load_w(dram, dst):
                    for fc in range(NFC):
                        nc.sync.dma_start(
                            out=dst[:, fc, :],
                            in_=dram[fc * 128:(fc + 1) * 128, :].bitcast(F32R))

                def load_x(ttg):
                    for fc in range(NFC):
                        nc.sync.dma_start(
                            out=xt[:, fc, ttg * 512:(ttg + 1) * 512],
                            in_=xqt[fc * 128:(fc + 1) * 128,
                                    ttg * 512:(ttg + 1) * 512].bitcast(F32R))

                # ordered so the first projections' inputs land first
                load_w(wq, wqb)
                load_x(0)
                load_x(1)
                load_w(wk, wkb)
                load_x(2)
                load_x(3)
                load_w(wv, wvb)

                def proj_q(hp):
                    for ttg in range(2):
                        ps = APs.tile([128, 512], F32, tag="projps",
                                      name="psq")
                        for fc in range(NFC):
                            nc.tensor.matmul(
                                ps[:],
                                wqb[:, fc, hp * 128:(hp + 1) * 128],
                                xt[:, fc, ttg * 512:(ttg + 1) * 512],
                                start=(fc == 0), stop=(fc == NFC - 1))
                        nc.vector.tensor_copy(
                            qt_[:, hp, ttg * 512:(ttg + 1) * 512], ps[:])

                def proj_k(hp):
                    for ttg in range(4):
                        ps = APs.tile([128, 512], F32, tag="projps",
                                      name="psk")
                        for fc in range(NFC):
                            nc.tensor.matmul(
                                ps[:],
                                wkb[:, fc, hp * 128:(hp + 1) * 128],
                                xt[:, fc, ttg * 512:(ttg + 1) * 512],
                                start=(fc == 0), stop=(fc == NFC - 1))
                        nc.vector.tensor_copy(
                            kt[:, hp, ttg * 512:(ttg + 1) * 512], ps[:])

                proj_q(0)
                proj_k(0)
                for gt in range(NGT):
                    ps = APs.tile([128, 512], F32, tag="projps", name="psv")
                    for fc in range(NFC):
                        nc.tensor.matmul(
                            ps[:],
                            xt[:, fc, gt * 128:(gt + 1) * 128],
                            wvb[:, fc, :],
                            start=(fc == 0), stop=(fc == NFC - 1))
                    nc.vector.tensor_copy(
                        v_[:, gt, :, 0:DK],
                        ps.rearrange("p (h v) -> p h v", h=H))
                for hp in range(1, NHP):
                    proj_q(hp)
                    proj_k(hp)

            with tc.tile_pool(name="late", bufs=1) as L:
                htn = L.tile([DK, H, NQ], F32R)   # normalized headsT
                dsum = L.tile([1, H, NQ], F32)    # 1/denominator rows
                wob = L.tile([DK, H * D], F32R)
                nc.sync.dma_start(out=wob[:], in_=wo[:, :].bitcast(F32R))

                # ---------------- phase B ----------------
                with tc.tile_pool(name="ub", bufs=4) as UB, \
                     tc.tile_pool(name="nrm", bufs=2) as NR, \
                     tc.tile_pool(name="spsp", bufs=2, space="PSUM") as SPs, \
                     tc.tile_pool(name="hvp", bufs=1, space="PSUM") as HVs:
                    for hp in range(NHP):
                        hv = [HVs.tile([DK + 1, NQ], F32, tag=f"hv{i}",
                                       name=f"hv{i}") for i in range(2)]
                        for gt in range(NGT):
                            for i in range(2):
                                h = hp * 2 + i
                                s = SPs.tile([128, NQ], F32, tag="sps")
                                for qg in range(2):
                                    nc.tensor.matmul(
                                        s[:, qg * 512:(qg + 1) * 512],
                                        kt[i * 64:(i + 1) * 64, hp,
                                           gt * 128:(gt + 1) * 128],
                                        qt_[i * 64:(i + 1) * 64, hp,
                                            qg * 512:(qg + 1) * 512],
                                        start=True, stop=True)
                                u = UB.tile([128, NQ], F16, tag="u")
                                nc.scalar.activation(u[:], s[:], EXP,
                                                     bias=nbias[:],
                                                     scale=NORM)
                                nc.vector.tensor_mul(u[:], u[:], nmt[:, gt, :])
                                for qg in range(2):
                                    nc.tensor.matmul(
                                        hv[i][:, qg * 512:(qg + 1) * 512],
                                        v_[:, gt, h, :],
                                        u[:, qg * 512:(qg + 1) * 512],
                                        start=(gt == 0), stop=(gt == NGT - 1))
                        # normalize straight out of PSUM
                        for i in range(2):
                            h = hp * 2 + i
                            nc.vector.tensor_copy(dsum[0:1, h, :],
                                                  hv[i][DK:DK + 1, :])
                            nc.vector.reciprocal_approx_fast(
                                dsum[0:1, h, :], dsum[0:1, h, :])
                            rinvb = NR.tile([DK, NQ], F32, tag="rinvb")
                            nc.gpsimd.partition_broadcast(rinvb[:],
                                                          dsum[0:1, h, :])
                            nc.vector.tensor_mul(htn[:, h, :],
                                                 hv[i][0:DK, :], rinvb[:])

                # ---------------- phase D: output projection ----------------
                with tc.tile_pool(name="pd", bufs=2) as DP, \
                     tc.tile_pool(name="pdps", bufs=2, space="PSUM") as DPs:
                    for qt in range(NQT):
                        po = DPs.tile([128, 512], F32, tag="po")
                        for h in range(H):
                            nc.tensor.matmul(
                                po[:],
                                htn[:, h, qt * 128:(qt + 1) * 128],
                                wob[:, h * D:(h + 1) * D],
                                start=(h == 0), stop=(h == H - 1))
                        ob = DP.tile([128, 512], F32, tag="ob")
                        nc.vector.tensor_copy(ob[:], po[:])
                        nc.sync.dma_start(
                            out=out[qt * 128:(qt + 1) * 128, :], in_=ob[:])

    nc.compile()
    _CACHE["nc"] = nc
    return nc


def kernel(q, mask, W_query, W_key, W_val, W_out):
    q = np.asarray(q, dtype=np.float32)
    mask = np.asarray(mask, dtype=np.int32)
    # [f, h*64+k] layouts for the projections, [k, h*512+e] for the output
    wq_r = np.ascontiguousarray(
        np.transpose(np.asarray(W_query, np.float32), (1, 0, 2)).reshape(D, D))
    wk_r = np.ascontiguousarray(
        np.transpose(np.asarray(W_key, np.float32), (1, 0, 2)).reshape(D, D))
    wv_r = np.ascontiguousarray(
        np.transpose(np.asarray(W_val, np.float32), (1, 0, 2)).reshape(D, D))
    wo_r = np.ascontiguousarray(
        np.transpose(np.asarray(W_out, np.float32), (1, 0, 2)).reshape(DK, H * D))

    nc = _build()
    in_maps = []
    for c in range(NCORES):
        b, qh = c // 2, c % 2
        xqt_c = q[b].T                                      # (D, N)
        nmt_c = 1.0 - mask[b, qh * NQ:(qh + 1) * NQ, :].T   # (N, NQ)
        if qh:
            # rotate the key axis so this core's queries are tokens [0, NQ)
            xqt_c = np.roll(xqt_c, -NQ, axis=1)
            nmt_c = np.roll(nmt_c, -NQ, axis=0)
        in_maps.append({
            "xqt": np.ascontiguousarray(xqt_c),
            "nmt": np.ascontiguousarray(nmt_c.astype(np.float16)),
            "wq": wq_r, "wk": wk_r, "wv": wv_r, "wo": wo_r,
        })
    res = run_bass_kernel_spmd(nc, in_maps, core_ids=list(range(NCORES)))
    output = np.empty((B, N, D), np.float32)
    for c in range(NCORES):
        b, qh = c // 2, c % 2
        output[b, qh * NQ:(qh + 1) * NQ, :] = res.results[c]["out"]
    return output


# revision 12
# speedup vs baseline: 11072.9716x; 10840.6894x over previous
"""Masked multi-head self-attention for Trainium2, SPMD over 8 NeuronCores.

Sharding: core c handles batch c//2, query-half c%2 (1024 of 2048 query rows).
The same Bass program runs on every core; odd cores get their inputs rotated
along the key axis so that "my" query rows are always tokens [0, 1024)
(attention sums are invariant to a consistent permutation of the key axis).

Host supplies x^T (features-major q) and (1-mask)^T in fp16, so the device
does no transposes. Per-core dataflow:
  Q^T/K^T (head-pair packed, fp32r) and V (token-major fp16, with a ones
        column for the softmax denominator) via PE projections from x^T
  S^T   = K @ Q^T per (head, key-tile) into fp32 PSUM (fp32r operands)
  U     = exp(0.125 * S^T - 8) on ACT (PSUM -> SBUF fp16), masked by
        (1-mask)^T via one DVE multiply
  headsT + denominator via PE (U moving, [V | 1] stationary, fp16)
  normalize straight out of PSUM per head-pair (reciprocal + GPSIMD
        partition-broadcast + DVE multiply -> fp32r headsT)
  output projection (fp32r) accumulated over heads in PSUM.
"""

import sys

sys.path.insert(0, "/opt/trn_rl_repo")

import numpy as np

import concourse.bass as bass  # noqa: F401
import concourse.tile as tile
from concourse import bacc, mybir
from concourse.bass_utils import run_bass_kernel_spmd

F32 = mybir.dt.float32
F32R = mybir.dt.float32r
F16 = mybir.dt.float16
EXP = mybir.ActivationFunctionType.Exp

B, N, D, H, DK = 4, 2048, 512, 8, 64
NQ = N // 2          # query rows per core
NORM = 1.0 / 8.0     # 1/sqrt(DK)
NFC = D // 128       # feature chunks (4)
NHP = H // 2         # head pairs (4)
NGT = N // 128       # key tiles (16)
NQT = NQ // 128      # query tiles per core (8)
NCORES = 8

_CACHE = {}


def _build():
    if "nc" in _CACHE:
        return _CACHE["nc"]
    nc = bacc.Bacc("TRN2", target_bir_lowering=False, debug=False,
                   num_devices=NCORES)
    xqt = nc.dram_tensor("xqt", [D, N], F32, kind="ExternalInput")
    nmtd = nc.dram_tensor("nmt", [N, NQ], F16, kind="ExternalInput")
    wq = nc.dram_tensor("wq", [D, D], F32, kind="ExternalInput")
    wk = nc.dram_tensor("wk", [D, D], F32, kind="ExternalInput")
    wv = nc.dram_tensor("wv", [D, D], F32, kind="ExternalInput")
    wo = nc.dram_tensor("wo", [DK, H * D], F32, kind="ExternalInput")
    out = nc.dram_tensor("out", [NQ, D], F32, kind="ExternalOutput")

    with tile.TileContext(nc) as tc:
        with tc.tile_pool(name="persist", bufs=1) as P:
            kt = P.tile([128, NHP, N], F32R)    # K^T two heads per tile
            qt_ = P.tile([128, NHP, NQ], F32R)  # Q^T two heads per tile
            v_ = P.tile([128, NGT, H, DK + 1], F16)  # V | ones
            nmt = P.tile([128, NGT, NQ], F16)        # (1-mask)^T
            nbias = P.tile([128, 1], F32)
            nc.vector.memset(nbias[:], -8.0)
            nc.vector.memset(v_[:, :, :, DK:DK + 1], 1.0)
            nmtv = nmtd.rearrange("(gc p) q -> p gc q", p=128)
            for gc in range(NGT):
                nc.sync.dma_start(out=nmt[:, gc, :], in_=nmtv[:, gc, :])

            # ---------------- phase A: loads + projections ----------------
            with tc.tile_pool(name="paps", bufs=3, space="PSUM") as APs, \
                 tc.tile_pool(name="xtp", bufs=1) as XT:
                xt = XT.tile([128, NFC, N], F32R)
                wqb = XT.tile([128, NFC, D], F32R)
                wkb = XT.tile([128, NFC, D], F32R)
                wvb = XT.tile([128, NFC, D], F32R)
                def load_w(dram, dst):
                    for fc in range(NFC):
                        nc.sync.dma_start(
                            out=dst[:, fc, :],
                            in_=dram[fc * 128:(fc + 1) * 128, :].bitcast(F32R))

                def load_x(ttg):
                    for fc in range(NFC):
                        nc.sync.dma_start(
                            out=xt[:, fc, ttg * 512:(ttg + 1) * 512],
                            in_=xqt[fc * 128:(fc + 1) * 128,
                                    ttg * 512:(ttg + 1) * 512].bitcast(F32R))

                # ordered so the first projections' inputs land first
                load_w(wq, wqb)
                load_x(0)
                load_x(1)
                load_w(wk, wkb)
                load_x(2)
                load_x(3)
                load_w(wv, wvb)

                def proj_q(hp):
                    for ttg in range(2):
                        ps = APs.tile([128, 512], F32, tag="projps",
                                      name="psq")
                        for fc in range(NFC):
                            nc.tensor.matmul(
                                ps[:],
                                wqb[:, fc, hp * 128:(hp + 1) * 128],
                                xt[:, fc, ttg * 512:(ttg + 1) * 512],
                                start=(fc == 0), stop=(fc == NFC - 1))
                        nc.vector.tensor_copy(
                            qt_[:, hp, ttg * 512:(ttg + 1) * 512], ps[:])

                def proj_k(hp):
                    for ttg in range(4):
                        ps = APs.tile([128, 512], F32, tag="projps",
                                      name="psk")
                        for fc in range(NFC):
                            nc.tensor.matmul(
                                ps[:],
                                wkb[:, fc, hp * 128:(hp + 1) * 128],
                                xt[:, fc, ttg * 512:(ttg + 1) * 512],
                                start=(fc == 0), stop=(fc == NFC - 1))
                        nc.vector.tensor_copy(
                            kt[:, hp, ttg * 512:(ttg + 1) * 512], ps[:])

                proj_q(0)
                proj_k(0)
                for gt in range(NGT):
                    ps = APs.tile([128, 512], F32, tag="projps", name="psv")
                    for fc in range(NFC):
                        nc.tensor.matmul(
                            ps[:],
                            xt[:, fc, gt * 128:(gt + 1) * 128],
                            wvb[:, fc, :],
                            start=(fc == 0), stop=(fc == NFC - 1))
                    nc.vector.tensor_copy(
                        v_[:, gt, :, 0:DK],
                        ps.rearrange("p (h v) -> p h v", h=H))
                for hp in range(1, NHP):
                    proj_q(hp)
                    proj_k(hp)

            with tc.tile_pool(name="late", bufs=1) as L:
                htn = L.tile([DK, H, NQ], F32R)   # normalized headsT
                dsum = L.tile([1, H, NQ], F32)    # 1/denominator rows
                wob = L.tile([DK, H * D], F32R)
                nc.sync.dma_start(out=wob[:], in_=wo[:, :].bitcast(F32R))

                # ---------------- phase B ----------------
                with tc.tile_pool(name="ub", bufs=4) as UB, \
                     tc.tile_pool(name="nrm", bufs=2) as NR, \
                     tc.tile_pool(name="spsp", bufs=2, space="PSUM") as SPs, \
                     tc.tile_pool(name="hvp", bufs=1, space="PSUM") as HVs:
                    for hp in range(NHP):
                        hv = [HVs.tile([DK + 1, NQ], F32, tag=f"hv{i}",
                                       name=f"hv{i}") for i in range(2)]
                        for gt in range(NGT):
                            for i in range(2):
                                h = hp * 2 + i
                                s = SPs.tile([128, NQ], F32, tag="sps")
                                for qg in range(2):
                                    nc.tensor.matmul(
                                        s[:, qg * 512:(qg + 1) * 512],
                                        kt[i * 64:(i + 1) * 64, hp,
                                           gt * 128:(gt + 1) * 128],
                                        qt_[i * 64:(i + 1) * 64, hp,
                                            qg * 512:(qg + 1) * 512],
                                        start=True, stop=True)
                                u = UB.tile([128, NQ], F16, tag="u")
                                nc.scalar.activation(u[:], s[:], EXP,
                                                     bias=nbias[:],
                                                     scale=NORM)
                                nc.vector.tensor_mul(u[:], u[:], nmt[:, gt, :])
                                for qg in range(2):
                                    nc.tensor.matmul(
                                        hv[i][:, qg * 512:(qg + 1) * 512],
                                        v_[:, gt, h, :],
                                        u[:, qg * 512:(qg + 1) * 512],
                                        start=(gt == 0), stop=(gt == NGT - 1))
                        # normalize straight out of PSUM
                        for i in range(2):
                            h = hp * 2 + i
                            nc.scalar.copy(dsum[0:1, h, :],
                                           hv[i][DK:DK + 1, :])
                            nc.vector.reciprocal_approx_fast(
                                dsum[0:1, h, :], dsum[0:1, h, :])
                            rinvb = NR.tile([DK, NQ], F32, tag="rinvb")
                            nc.gpsimd.partition_broadcast(rinvb[:],
                                                          dsum[0:1, h, :])
                            nc.vector.tensor_mul(htn[:, h, :],
                                                 hv[i][0:DK, :], rinvb[:])

                # ---------------- phase D: output projection ----------------
                with tc.tile_pool(name="pd", bufs=2) as DP, \
                     tc.tile_pool(name="pdps", bufs=2, space="PSUM") as DPs:
                    for qt in range(NQT):
                        po = DPs.tile([128, 512], F32, tag="po")
                        for h in range(H):
                            nc.tensor.matmul(
                                po[:],
                                htn[:, h, qt * 128:(qt + 1) * 128],
                                wob[:, h * D:(h + 1) * D],
                                start=(h == 0), stop=(h == H - 1))
                        ob = DP.tile([128, 512], F32, tag="ob")
                        nc.vector.tensor_copy(ob[:], po[:])
                        nc.sync.dma_start(
                            out=out[qt * 128:(qt + 1) * 128, :], in_=ob[:])

    nc.compile()
    _CACHE["nc"] = nc
    return nc


def kernel(q, mask, W_query, W_key, W_val, W_out):
    q = np.asarray(q, dtype=np.float32)
    mask = np.asarray(mask, dtype=np.int32)
    # [f, h*64+k] layouts for the projections, [k, h*512+e] for the output
    wq_r = np.ascontiguousarray(
        np.transpose(np.asarray(W_query, np.float32), (1, 0, 2)).reshape(D, D))
    wk_r = np.ascontiguousarray(
        np.transpose(np.asarray(W_key, np.float32), (1, 0, 2)).reshape(D, D))
    wv_r = np.ascontiguousarray(
        np.transpose(np.asarray(W_val, np.float32), (1, 0, 2)).reshape(D, D))
    wo_r = np.ascontiguousarray(
        np.transpose(np.asarray(W_out, np.float32), (1, 0, 2)).reshape(DK, H * D))

    nc = _build()
    in_maps = []
    for c in range(NCORES):
        b, qh = c // 2, c % 2
        xqt_c = q[b].T                                      # (D, N)
        nmt_c = 1.0 - mask[b, qh * NQ:(qh + 1) * NQ, :].T   # (N, NQ)
        if qh:
            # rotate the key axis so this core's queries are tokens [0, NQ)
            xqt_c = np.roll(xqt_c, -NQ, axis=1)
            nmt_c = np.roll(nmt_c, -NQ, axis=0)
        in_maps.append({
            "xqt": np.ascontiguousarray(xqt_c),
            "nmt": np.ascontiguousarray(nmt_c.astype(np.float16)),
            "wq": wq_r, "wk": wk_r, "wv": wv_r, "wo": wo_r,
        })
    res = run_bass_kernel_spmd(nc, in_maps, core_ids=list(range(NCORES)))
    output = np.empty((B, N, D), np.float32)
    for c in range(NCORES):
        b, qh = c // 2, c % 2
        output[b, qh * NQ:(qh + 1) * NQ, :] = res.results[c]["out"]
    return output
